# revision 35
# baseline (speedup 1.0000x reference)
"""Trainium2 Bass kernel for batched multi-head graph attention (GAT).

Reference computation (per batch b, head h):
    h_prime = h[b] @ w[h]                      # [N, FOUT]
    t = tanh(h_prime)
    src = t @ a_src[h]; dst = t @ a_dst[h]     # [N]
    s[i,j] = leaky_relu_{0.2}(src[i] + dst[j])
    attn = softmax_j(where(adj[b]>0, s, -inf))
    out[b,h] = attn @ h_prime

Device algorithm (core c <-> batch b=c):
    exp(leaky_relu(s)) = max(e^s, e^{0.2 s}); with s = src_i + dst_j the
    unnormalized weight factors (dropping the e^{src_i} row factor,
    which cancels in the softmax) as
        W[j,i] = adjT[j,i] * max(u_i * vq_j, q_j)
    with u = e^{-0.8 src}, vq = e^{0.2 dst}, q = e^{dst}.
    Per 128-row chunk of the score matrix the mask work is split across
    engines to balance the three pipelines:
      - "classic" chunks (DVE): mx = tensor_scalar(ub, *vq_j, *q_j,
        mult, max), then z = tensor_tensor(mx, adjT, mult).
      - "relu" chunks (ACT + PE): max(a, q) = q + relu(a - q), so the
        scalar engine computes rel = relu(vq_j * ub - q_j) (per-
        partition scale/bias), the DVE only does the mask multiply
        z = rel * adjT, and the PE accumulates the base term
        hp1q^T @ adjT (hp1q = hp1 * q_j, its ones column becoming the
        q_j denominator term) into the same PSUM accumulator.
    ub (u_i broadcast to 128 partitions) comes from one PE matmul with
    a host-replicated a_src stationary + ACT exp straight out of PSUM.
    The PE contracts outT[f,i] += hp1[j,f] * z[j,i]; the ones column
    accumulates the softmax denominator. An ungated warm-up burst at
    t=0 lifts the PE HAM clock to 2.4 GHz during the DMA prefix; tiny
    dummy matmuls paced between the mask chunks keep it there. The host
    divides rows 0..63 by row 64 (fp16 output) and transposes to
    [b, h, n, f].
"""

import numpy as np
import ml_dtypes

import concourse.mybir as mybir
import concourse.tile as tile
from concourse import bacc
from concourse.bass_utils import run_bass_kernel_spmd

BS, N, FIN, NH, FOUT = 8, 1024, 256, 8, 64
P = 128
NCH = N // P          # 8 chunks of the node axis
KC = FIN // P         # 2 chunks of the feature-in axis
NPAIR = NH // 2       # head pairs packed into 128 partitions
FP = FOUT + 2         # hp1 row width: 64 features + ones col + pad (even)
F32 = mybir.dt.float32
BF16 = mybir.dt.bfloat16
F16 = mybir.dt.float16
AX = mybir.AxisListType
ALU = mybir.AluOpType
ACTF = mybir.ActivationFunctionType
BF16NP = ml_dtypes.bfloat16

N_RELU = 3            # chunks 8-N_RELU..7 of each head take the ACT+PE path


def emit(nc, tc, hT_d, w_d, aPC_d, aRep_d, adjT_d, out_d):
    with (
        tc.tile_pool(name="const", bufs=1) as cpool,
        tc.tile_pool(name="ub", bufs=3) as ubpool,
        tc.tile_pool(name="mx", bufs=3) as mxpool,
        tc.tile_pool(name="z", bufs=3) as zpool,
        tc.tile_pool(name="osb", bufs=2) as opool,
        tc.tile_pool(name="psa", bufs=2, space="PSUM") as pp_a,
        tc.tile_pool(name="psb", bufs=2, space="PSUM") as pp_b,
        tc.tile_pool(name="psout", bufs=4, space="PSUM") as pp_out,
    ):
        # ---- ungated PE warm-up: memset a dummy row, then ~4us of
        # N=512 matmuls so the HAM clock is 8/8 before real work ----
        dummy = cpool.tile([1, 512], BF16)
        nc.vector.memset(dummy, 1.0)
        dumo = cpool.tile([1, 512], BF16)
        # preload the exp/tanh/relu spline table set during the DMA shadow
        nc.scalar.activation(dumo, dummy, ACTF.Exp)
        for _ in range(8):
            wps = pp_b.tile([1, 512], F32, tag="b", name="wps")
            nc.tensor.matmul(
                wps, dummy[:, 0:1], dummy,
                start=True, stop=True, skip_group_check=True,
            )

        # ---- constant loads. hT is the critical-path input: issue it
        # first, split so it spreads across many DMA engines. adjT (bulk,
        # needed later) issues from the otherwise-idle GpSimd queue. ----
        hT = cpool.tile([P, KC, N], BF16)
        wsb = cpool.tile([P, KC, NH * FOUT], BF16)
        aPC = cpool.tile([P, NPAIR, 4], BF16)
        aRep = cpool.tile([P, NPAIR, 2, P], BF16)
        adjT = cpool.tile([P, NCH, N], BF16)
        # Each dma_start streams at only ~50-80 GB/s, but streams run
        # concurrently: split the critical-path inputs into many small
        # issues spread over both the sync and gpsimd queues, ordered by
        # when the compute needs them.
        nc.sync.dma_start(hT[:, :, 0:256], hT_d[0][:, :, 0:256])
        nc.sync.dma_start(hT[:, :, 256:512], hT_d[0][:, :, 256:512])
        nc.gpsimd.dma_start(wsb[:, 0, :], w_d[:, 0, :])
        nc.gpsimd.dma_start(wsb[:, 1, :], w_d[:, 1, :])
        nc.gpsimd.dma_start(aPC, aPC_d)
        nc.gpsimd.dma_start(hT[:, :, 512:768], hT_d[1][:, :, 0:256])
        nc.gpsimd.dma_start(hT[:, :, 768:1024], hT_d[1][:, :, 256:512])
        nc.gpsimd.dma_start(aRep, aRep_d)
        for jc in range(NCH):
            nc.sync.dma_start(adjT[:, jc, :], adjT_d[:, jc, :])

        # hp1[:, ic, h, 0:64] = h_prime, col 64 = ones (softmax
        # denominator), col 65 = zero pad; hp1q = hp1 * q_j.
        hp1 = cpool.tile([P, NCH, NH, FP], BF16)
        nc.vector.memset(hp1[:, :, :, FOUT:FP], 0.0)
        nc.vector.memset(hp1[:, :, :, FOUT : FOUT + 1], 1.0)
        hp1q = cpool.tile([P, NCH, NH, FP], BF16)
        tT = cpool.tile([P, NPAIR, N], BF16)
        q_col = cpool.tile([P, NPAIR, NCH, 2], F32)
        nq_col = cpool.tile([P, NPAIR, NCH, 2], F32)
        vq_col = cpool.tile([P, NPAIR, NCH, 2], F32)

        def emit_pair(p):
            """hpT = w^T h for head pair p, then tanh."""
            for half in range(2):
                hpT = pp_b.tile([P, 512], F32, tag="b", name="hpT")
                for kc in range(KC):
                    nc.tensor.matmul(
                        hpT,
                        wsb[:, kc, 2 * p * FOUT : (2 * p + 2) * FOUT],
                        hT[:, kc, half * 512 : (half + 1) * 512],
                        start=(kc == 0),
                        stop=(kc == KC - 1),
                    )
                nc.scalar.activation(
                    tT[:, p, half * 512 : (half + 1) * 512], hpT, ACTF.Tanh
                )

        def emit_dcol(p):
            """dst-col projections -> q = e^dst, vq = e^{0.2 dst}, -q.
            Emitted after the first ub of the pair so the ub chain (the
            critical path into the DVE) gets the PE/ACT queues first."""
            dcol = pp_a.tile([P, NCH * 2], F32, tag="a", name="dcol")
            for jc in range(NCH):
                nc.tensor.matmul(
                    dcol[:, 2 * jc : 2 * jc + 2],
                    tT[:, p, jc * P : (jc + 1) * P],
                    aPC[:, p, 2:4],
                    start=True,
                    stop=True,
                )
            nc.scalar.activation(
                q_col[:, p, :, :], dcol.rearrange("p (j c) -> p j c", c=2), ACTF.Exp
            )
            nc.scalar.activation(
                vq_col[:, p, :, :], dcol.rearrange("p (j c) -> p j c", c=2),
                ACTF.Exp, scale=0.2,
            )
            nc.vector.tensor_scalar(
                nq_col[:, p, :, :], q_col[:, p, :, :], -1.0, None, ALU.mult
            )

        def emit_ub(h):
            """ub[j, i] = e^{-0.8 src_i} on all 128 partitions: one PE
            matmul vs the replicated a_src stationary, exp from PSUM."""
            p, h2 = h // 2, h % 2
            ub = ubpool.tile([P, N], BF16, name=f"ub{h}")
            for half in range(2):
                ubps = pp_b.tile([P, 512], F32, tag="b", name="ubps")
                nc.tensor.matmul(
                    ubps,
                    aRep[:, p, h2, :],
                    tT[:, p, half * 512 : (half + 1) * 512],
                    start=True, stop=True,
                )
                nc.scalar.activation(
                    ub[:, half * 512 : (half + 1) * 512], ubps,
                    ACTF.Exp, scale=-0.8,
                )
            return ub

        def emit_hp1q(p):
            """hp1q[:, ic, h, :] = hp1 * q for the relu chunks only (ones
            col becomes the q_j denominator column of the base term)."""
            for h2 in range(2):
                h = 2 * p + h2
                for ic in range(NCH - N_RELU, NCH):
                    nc.vector.tensor_scalar(
                        hp1q[:, ic, h, :], hp1[:, ic, h, :],
                        q_col[:, p, ic, h2 : h2 + 1], None, ALU.mult,
                    )

        def warm_pe(pso):
            # Tiny matmul into the unused col-group 3 / partition 96 of the
            # live pso bank: resets the PE HAM idle window during
            # vector-bound stretches so real matmuls stay at 2.4 GHz.
            nc.tensor.matmul(
                pso[96:97, 0:8],
                hp1[:, 0, 0, 0:1],
                adjT[:, 0, 0:8],
                start=True, stop=True,
                tile_position=(0, 96),
                skip_group_check=True,
            )

        def emit_head(h, ub):
            """Masked unnormalized weights + attention matmuls for head h.
            Heads 0-1 run all-classic: their relu path would need hp1q
            before the pipeline has warmed up, stalling both queues."""
            p, h2 = h // 2, h % 2
            relu_jcs = set(range(NCH - N_RELU, NCH)) if h >= 2 else set()
            pso = [
                pp_out.tile([P, 512], F32, tag="out", name=f"pso{half}")
                for half in range(2)
            ]
            # base term of the relu chunks: hp1q^T @ adjT, accumulated
            # first so the PE works while the DVE/ACT produce the masks
            first = True
            for jc in sorted(relu_jcs):
                for half in range(2):
                    nc.tensor.matmul(
                        pso[half][0:FP, :],
                        hp1q[:, jc, h, :],
                        adjT[:, jc, half * 512 : (half + 1) * 512],
                        start=first,
                        stop=False,
                    )
                first = False
            for jc0 in (0, 4):
                mx = mxpool.tile([P, 4, N], BF16)
                for k in range(4):
                    jc = jc0 + k
                    if jc in relu_jcs:
                        nc.scalar.activation(
                            mx[:, k, :], ub, ACTF.Relu,
                            bias=nq_col[:, p, jc, h2 : h2 + 1],
                            scale=vq_col[:, p, jc, h2 : h2 + 1],
                        )
                    else:
                        nc.vector.tensor_scalar(
                            mx[:, k, :], ub,
                            vq_col[:, p, jc, h2 : h2 + 1],
                            q_col[:, p, jc, h2 : h2 + 1],
                            ALU.mult, ALU.max,
                        )
                    if k == 2 or (h == NH - 1 and k == 3):
                        warm_pe(pso[0])
                z = zpool.tile([P, 4, N], BF16)
                if h == NH - 1 and jc0 == 4:
                    # split the last mask multiply so the final attention
                    # matmuls (and the output tail) start ~1us earlier
                    nc.vector.tensor_tensor(
                        z[:, 0:2, :], mx[:, 0:2, :], adjT[:, 4:6, :], ALU.mult
                    )
                    nc.vector.tensor_tensor(
                        z[:, 2:4, :], mx[:, 2:4, :], adjT[:, 6:8, :], ALU.mult
                    )
                else:
                    nc.vector.tensor_tensor(
                        z, mx, adjT[:, jc0 : jc0 + 4, :], ALU.mult
                    )
                # half-major in the last group so the half-0 output copy can
                # start while half-1 matmuls still run
                order = (
                    [(k, half) for k in range(4) for half in range(2)]
                    if jc0 == 0
                    else [(k, half) for half in range(2) for k in range(4)]
                )
                for k, half in order:
                    jc = jc0 + k
                    nc.tensor.matmul(
                        pso[half][0:FP, :],
                        hp1[:, jc, h, :],
                        z[:, k, half * 512 : (half + 1) * 512],
                        start=(jc == 0 and not relu_jcs),
                        stop=(jc == NCH - 1),
                    )
            ot = opool.tile([FOUT + 1, N], F16)
            for half in range(2):
                nc.scalar.activation(
                    ot[:, half * 512 : (half + 1) * 512],
                    pso[half][0 : FOUT + 1, :], ACTF.Copy,
                )
                eng = nc.gpsimd if h == NH - 1 else nc.sync
                eng.dma_start(
                    out_d[h][:, half * 512 : (half + 1) * 512],
                    ot[:, half * 512 : (half + 1) * 512],
                )

        def emit_phase_a(ic0, ic1, on_dve=False):
            # h_prime in [node, head*fout] layout for attention stationaries.
            # With on_dve the PSUM->SBUF copy runs on the vector engine --
            # used for the early chunks, which land in the prefix window
            # where the DVE is otherwise idle, freeing the scalar queue.
            for ic in range(ic0, ic1):
                ps = pp_a.tile([P, NH * FOUT], F32, tag="a", name="ps")
                for kc in range(KC):
                    nc.tensor.matmul(
                        ps,
                        hT[:, kc, ic * P : (ic + 1) * P],
                        wsb[:, kc, :],
                        start=(kc == 0),
                        stop=(kc == KC - 1),
                    )
                if on_dve:
                    nc.vector.tensor_copy(
                        hp1[:, ic, :, 0:FOUT],
                        ps.rearrange("p (h f) -> p h f", f=FOUT),
                    )
                else:
                    nc.scalar.activation(
                        hp1[:, ic, :, 0:FOUT],
                        ps.rearrange("p (h f) -> p h f", f=FOUT),
                        ACTF.Copy,
                    )

        # ---- software-pipelined emission ----
        # Fill window: the critical chain into the first mask op is
        # pair 0 -> ub(0) -> dcol(0); everything else here soaks up the
        # otherwise-idle PE/ACT/DVE time while the DMAs land.
        emit_pair(0)
        ubs = {0: emit_ub(0)}
        emit_dcol(0)
        emit_phase_a(0, 4, on_dve=True)
        emit_phase_a(4, NCH)
        emit_pair(1)
        emit_dcol(1)
        emit_pair(2)
        emit_dcol(2)
        emit_pair(3)
        emit_dcol(3)
        for h in range(NH):
            # keep the next head's inputs ready while the vector engine
            # grinds the current head's mask chunks
            if h + 1 < NH:
                ubs[h + 1] = emit_ub(h + 1)
                if h + 1 in (2, 4, 6):
                    emit_hp1q((h + 1) // 2)
            emit_head(h, ubs.pop(h))


def build_program(num_devices=8, debug=False):
    nc = bacc.Bacc(
        "TRN2", target_bir_lowering=False, debug=debug, num_devices=num_devices
    )
    hT_d = nc.dram_tensor("hT", [2, P, KC, 512], BF16, kind="ExternalInput").ap()
    w_d = nc.dram_tensor("w_all", [P, KC, NH * FOUT], BF16, kind="ExternalInput").ap()
    aPC_d = nc.dram_tensor("aPC", [P, NPAIR, 4], BF16, kind="ExternalInput").ap()
    aRep_d = nc.dram_tensor("aRep", [P, NPAIR, 2, P], BF16, kind="ExternalInput").ap()
    adjT_d = nc.dram_tensor("adjT", [P, NCH, N], BF16, kind="ExternalInput").ap()
    out_d = nc.dram_tensor("outT", [NH, FOUT + 1, N], F16, kind="ExternalOutput").ap()
    with tile.TileContext(nc) as tc:
        emit(nc, tc, hT_d, w_d, aPC_d, aRep_d, adjT_d, out_d)
    nc.compile()
    return nc


def make_in_maps(h, adj, w, a_src, a_dst):
    """Host-side sharding/layout prep: core c gets batch c."""
    # All DRAM tensors are laid out partition-major so every DMA is a
    # contiguous multi-KB run per partition (strided DMAs collapse to
    # ~512B packets and starve the queues).
    w_all = np.ascontiguousarray(
        w.astype(np.float32)
        .transpose(1, 0, 2)
        .reshape(KC, P, NH * FOUT)
        .transpose(1, 0, 2)
    ).astype(BF16NP)
    # aPC[p]: [128, 4] = (src_A, src_B, dst_A, dst_B) columns for head pair
    # (2p, 2p+1); head A occupies partition rows 0:64, head B rows 64:128.
    aPC = np.zeros((NPAIR, P, 4), dtype=np.float32)
    for p in range(NPAIR):
        aPC[p, 0:FOUT, 0] = a_src[2 * p, :, 0]
        aPC[p, FOUT:P, 1] = a_src[2 * p + 1, :, 0]
        aPC[p, 0:FOUT, 2] = a_dst[2 * p, :, 0]
        aPC[p, FOUT:P, 3] = a_dst[2 * p + 1, :, 0]
    # aRep[f, p, h2, m] = aPC src column, replicated over the 128 moving
    # columns: the stationary that broadcasts src rows to all partitions.
    aRep = np.ascontiguousarray(
        np.broadcast_to(
            aPC[:, :, 0:2].transpose(1, 0, 2)[:, :, :, None], (P, NPAIR, 2, P)
        )
    ).astype(BF16NP)
    aPC = np.ascontiguousarray(aPC.transpose(1, 0, 2)).astype(BF16NP)
    in_maps = []
    for b in range(BS):
        hTkpn = h[b].astype(np.float32).T.reshape(KC, P, N).transpose(1, 0, 2)
        hTb = np.ascontiguousarray(
            hTkpn.reshape(P, KC, 2, 512).transpose(2, 0, 1, 3)
        ).astype(BF16NP)
        adjTb = np.ascontiguousarray(
            adj[b].T.reshape(NCH, P, N).transpose(1, 0, 2)
        ).astype(BF16NP)
        in_maps.append(
            {"hT": hTb, "w_all": w_all, "aPC": aPC, "aRep": aRep, "adjT": adjTb}
        )
    return in_maps


def postprocess(raw_outs):
    """raw_outs: list of [NH, FOUT+1, N] per core -> full [BS, NH, N, FOUT]."""
    outT = np.stack([np.asarray(r, dtype=np.float32) for r in raw_outs])
    num = outT[:, :, 0:FOUT, :]
    den = outT[:, :, FOUT : FOUT + 1, :]
    return np.ascontiguousarray((num / den).transpose(0, 1, 3, 2)).astype(np.float32)


_NC_CACHE = {}


def kernel(h, adj, w, a_src, a_dst):
    if "nc" not in _NC_CACHE:
        _NC_CACHE["nc"] = build_program(num_devices=BS)
    nc = _NC_CACHE["nc"]
    in_maps = make_in_maps(h, adj, w, a_src, a_dst)
    res = run_bass_kernel_spmd(nc, in_maps, core_ids=list(range(BS)))
    return postprocess([r["outT"] for r in res.results])


# revision 36
# speedup vs baseline: 1.0567x; 1.0567x over previous
"""Trainium2 Bass kernel for batched multi-head graph attention (GAT).

Reference computation (per batch b, head h):
    h_prime = h[b] @ w[h]                      # [N, FOUT]
    t = tanh(h_prime)
    src = t @ a_src[h]; dst = t @ a_dst[h]     # [N]
    s[i,j] = leaky_relu_{0.2}(src[i] + dst[j])
    attn = softmax_j(where(adj[b]>0, s, -inf))
    out[b,h] = attn @ h_prime

Device algorithm (core c <-> batch b=c):
    exp(leaky_relu(s)) = max(e^s, e^{0.2 s}); with s = src_i + dst_j the
    unnormalized weight factors (dropping the e^{src_i} row factor,
    which cancels in the softmax) as
        W[j,i] = adjT[j,i] * max(u_i * vq_j, q_j)
    with u = e^{-0.8 src}, vq = e^{0.2 dst}, q = e^{dst}.
    Per 128-row chunk of the score matrix the mask work is split across
    engines to balance the three pipelines:
      - "classic" chunks (DVE): mx = tensor_scalar(ub, *vq_j, *q_j,
        mult, max), then z = tensor_tensor(mx, adjT, mult).
      - "relu" chunks (ACT + PE): max(a, q) = q + relu(a - q), so the
        scalar engine computes rel = relu(vq_j * ub - q_j) (per-
        partition scale/bias), the DVE only does the mask multiply
        z = rel * adjT, and the PE accumulates the base term
        hp1q^T @ adjT (hp1q = hp1 * q_j, its ones column becoming the
        q_j denominator term) into the same PSUM accumulator.
    ub (u_i broadcast to 128 partitions) comes from one PE matmul with
    a host-replicated a_src stationary + ACT exp straight out of PSUM.
    The PE contracts outT[f,i] += hp1[j,f] * z[j,i]; the ones column
    accumulates the softmax denominator. An ungated warm-up burst at
    t=0 lifts the PE HAM clock to 2.4 GHz during the DMA prefix; tiny
    dummy matmuls paced between the mask chunks keep it there. The host
    divides rows 0..63 by row 64 (fp16 output) and transposes to
    [b, h, n, f].
"""

import numpy as np
import ml_dtypes

import concourse.mybir as mybir
import concourse.tile as tile
from concourse import bacc
from concourse.bass_utils import run_bass_kernel_spmd

BS, N, FIN, NH, FOUT = 8, 1024, 256, 8, 64
P = 128
NCH = N // P          # 8 chunks of the node axis
KC = FIN // P         # 2 chunks of the feature-in axis
NPAIR = NH // 2       # head pairs packed into 128 partitions
FP = FOUT + 2         # hp1 row width: 64 features + ones col + pad (even)
F32 = mybir.dt.float32
BF16 = mybir.dt.bfloat16
F16 = mybir.dt.float16
AX = mybir.AxisListType
ALU = mybir.AluOpType
ACTF = mybir.ActivationFunctionType
BF16NP = ml_dtypes.bfloat16

N_RELU = 3            # chunks 8-N_RELU..7 of each head take the ACT+PE path


def emit(nc, tc, hT_d, w_d, aPC_d, aRep_d, adjT_d, out_d):
    with (
        tc.tile_pool(name="const", bufs=1) as cpool,
        tc.tile_pool(name="ub", bufs=3) as ubpool,
        tc.tile_pool(name="mx", bufs=3) as mxpool,
        tc.tile_pool(name="z", bufs=3) as zpool,
        tc.tile_pool(name="osb", bufs=2) as opool,
        tc.tile_pool(name="psa", bufs=2, space="PSUM") as pp_a,
        tc.tile_pool(name="psb", bufs=2, space="PSUM") as pp_b,
        tc.tile_pool(name="psout", bufs=4, space="PSUM") as pp_out,
    ):
        # ---- ungated PE warm-up: memset a dummy row, then N=512
        # matmuls that bridge until the inputs land ----
        dummy = cpool.tile([1, 512], BF16)
        nc.vector.memset(dummy, 1.0)
        for _ in range(12):
            wps = pp_b.tile([1, 512], F32, tag="b", name="wps")
            nc.tensor.matmul(
                wps, dummy[:, 0:1], dummy,
                start=True, stop=True, skip_group_check=True,
            )

        # ---- constant loads (small tensors first so compute can start
        # while the bulk adjacency is still in flight) ----
        hT = cpool.tile([P, KC, N], BF16)
        wsb = cpool.tile([P, KC, NH * FOUT], BF16)
        aPC = cpool.tile([P, NPAIR, 4], BF16)
        aRep = cpool.tile([P, NPAIR, 2, P], BF16)
        adjT = cpool.tile([P, NCH, N], BF16)
        nc.sync.dma_start(wsb, w_d.rearrange("k p c -> p k c"))
        nc.sync.dma_start(aPC, aPC_d.rearrange("q p c -> p q c"))
        nc.sync.dma_start(aRep, aRep_d)
        for hf in range(2):
            nc.sync.dma_start(
                hT[:, :, hf * 512 : (hf + 1) * 512],
                hT_d[:, :, hf * 512 : (hf + 1) * 512].rearrange("k p c -> p k c"),
            )
        for jc in range(NCH):
            nc.sync.dma_start(adjT[:, jc, :], adjT_d[jc])

        # hp1[:, ic, h, 0:64] = h_prime, col 64 = ones (softmax
        # denominator), col 65 = zero pad; hp1q = hp1 * q_j.
        hp1 = cpool.tile([P, NCH, NH, FP], BF16)
        nc.vector.memset(hp1[:, :, :, FOUT:FP], 0.0)
        nc.vector.memset(hp1[:, :, :, FOUT : FOUT + 1], 1.0)
        hp1q = cpool.tile([P, NCH, NH, FP], BF16)
        tT = cpool.tile([P, NPAIR, N], BF16)
        q_col = cpool.tile([P, NPAIR, NCH, 2], F32)
        nq_col = cpool.tile([P, NPAIR, NCH, 2], F32)
        vq_col = cpool.tile([P, NPAIR, NCH, 2], F32)

        def emit_pair(p):
            """hpT = w^T h for head pair p, tanh, then dst-col
            projections on the PE -> q = e^dst, vq = e^{0.2 dst}."""
            for half in range(2):
                hpT = pp_b.tile([P, 512], F32, tag="b", name="hpT")
                for kc in range(KC):
                    nc.tensor.matmul(
                        hpT,
                        wsb[:, kc, 2 * p * FOUT : (2 * p + 2) * FOUT],
                        hT[:, kc, half * 512 : (half + 1) * 512],
                        start=(kc == 0),
                        stop=(kc == KC - 1),
                    )
                nc.scalar.activation(
                    tT[:, p, half * 512 : (half + 1) * 512], hpT, ACTF.Tanh
                )
            # dst cols: [128, 2] per jc = tT_chunk^T @ aDstCols
            dcol = pp_a.tile([P, NCH * 2], F32, tag="a", name="dcol")
            for jc in range(NCH):
                nc.tensor.matmul(
                    dcol[:, 2 * jc : 2 * jc + 2],
                    tT[:, p, jc * P : (jc + 1) * P],
                    aPC[:, p, 2:4],
                    start=True,
                    stop=True,
                )
            nc.scalar.activation(
                q_col[:, p, :, :], dcol.rearrange("p (j c) -> p j c", c=2), ACTF.Exp
            )
            nc.scalar.activation(
                vq_col[:, p, :, :], dcol.rearrange("p (j c) -> p j c", c=2),
                ACTF.Exp, scale=0.2,
            )
            nc.vector.tensor_scalar(
                nq_col[:, p, :, :], q_col[:, p, :, :], -1.0, None, ALU.mult
            )

        def emit_ub(h):
            """ub[j, i] = e^{-0.8 src_i} on all 128 partitions: one PE
            matmul vs the replicated a_src stationary, exp from PSUM."""
            p, h2 = h // 2, h % 2
            ub = ubpool.tile([P, N], BF16, name=f"ub{h}")
            for half in range(2):
                ubps = pp_b.tile([P, 512], F32, tag="b", name="ubps")
                nc.tensor.matmul(
                    ubps,
                    aRep[:, p, h2, :],
                    tT[:, p, half * 512 : (half + 1) * 512],
                    start=True, stop=True,
                )
                nc.scalar.activation(
                    ub[:, half * 512 : (half + 1) * 512], ubps,
                    ACTF.Exp, scale=-0.8,
                )
            return ub

        def emit_hp1q(p):
            """hp1q[:, ic, h, :] = hp1 * q for the relu chunks only (ones
            col becomes the q_j denominator column of the base term)."""
            for h2 in range(2):
                h = 2 * p + h2
                for ic in range(NCH - N_RELU, NCH):
                    nc.vector.tensor_scalar(
                        hp1q[:, ic, h, :], hp1[:, ic, h, :],
                        q_col[:, p, ic, h2 : h2 + 1], None, ALU.mult,
                    )

        def warm_pe(pso):
            # Tiny matmul into the unused col-group 3 / partition 96 of the
            # live pso bank: resets the PE HAM idle window during
            # vector-bound stretches so real matmuls stay at 2.4 GHz.
            nc.tensor.matmul(
                pso[96:97, 0:8],
                hp1[:, 0, 0, 0:1],
                adjT[:, 0, 0:8],
                start=True, stop=True,
                tile_position=(0, 96),
                skip_group_check=True,
            )

        def emit_head(h, ub):
            """Masked unnormalized weights + attention matmuls for head h."""
            p, h2 = h // 2, h % 2
            relu_jcs = set(range(NCH - N_RELU, NCH))
            pso = [
                pp_out.tile([P, 512], F32, tag="out", name=f"pso{half}")
                for half in range(2)
            ]
            # base term of the relu chunks: hp1q^T @ adjT, accumulated
            # first so the PE works while the DVE/ACT produce the masks
            first = True
            for jc in sorted(relu_jcs):
                for half in range(2):
                    nc.tensor.matmul(
                        pso[half][0:FP, :],
                        hp1q[:, jc, h, :],
                        adjT[:, jc, half * 512 : (half + 1) * 512],
                        start=first,
                        stop=False,
                    )
                first = False
            for jc0 in (0, 4):
                mx = mxpool.tile([P, 4, N], BF16)
                for k in range(4):
                    jc = jc0 + k
                    if jc in relu_jcs:
                        nc.scalar.activation(
                            mx[:, k, :], ub, ACTF.Relu,
                            bias=nq_col[:, p, jc, h2 : h2 + 1],
                            scale=vq_col[:, p, jc, h2 : h2 + 1],
                        )
                    else:
                        nc.vector.tensor_scalar(
                            mx[:, k, :], ub,
                            vq_col[:, p, jc, h2 : h2 + 1],
                            q_col[:, p, jc, h2 : h2 + 1],
                            ALU.mult, ALU.max,
                        )
                    if k == 2:
                        warm_pe(pso[0])
                z = zpool.tile([P, 4, N], BF16)
                nc.vector.tensor_tensor(
                    z, mx, adjT[:, jc0 : jc0 + 4, :], ALU.mult
                )
                # half-major in the last group so the half-0 output copy can
                # start while half-1 matmuls still run
                order = (
                    [(k, half) for k in range(4) for half in range(2)]
                    if jc0 == 0
                    else [(k, half) for half in range(2) for k in range(4)]
                )
                for k, half in order:
                    jc = jc0 + k
                    nc.tensor.matmul(
                        pso[half][0:FP, :],
                        hp1[:, jc, h, :],
                        z[:, k, half * 512 : (half + 1) * 512],
                        start=(jc == 0 and not relu_jcs),
                        stop=(jc == NCH - 1),
                    )
            ot = opool.tile([FOUT + 1, N], F16)
            for half in range(2):
                nc.scalar.activation(
                    ot[:, half * 512 : (half + 1) * 512],
                    pso[half][0 : FOUT + 1, :], ACTF.Copy,
                )
                nc.sync.dma_start(
                    out_d[h][:, half * 512 : (half + 1) * 512],
                    ot[:, half * 512 : (half + 1) * 512],
                )

        def emit_phase_a(ic0, ic1, on_dve=False):
            # h_prime in [node, head*fout] layout for attention stationaries.
            # With on_dve the PSUM->SBUF copy runs on the vector engine --
            # used for the early chunks, which land in the prefix window
            # where the DVE is otherwise idle, freeing the scalar queue.
            for ic in range(ic0, ic1):
                ps = pp_a.tile([P, NH * FOUT], F32, tag="a", name="ps")
                for kc in range(KC):
                    nc.tensor.matmul(
                        ps,
                        hT[:, kc, ic * P : (ic + 1) * P],
                        wsb[:, kc, :],
                        start=(kc == 0),
                        stop=(kc == KC - 1),
                    )
                if on_dve:
                    nc.vector.tensor_copy(
                        hp1[:, ic, :, 0:FOUT],
                        ps.rearrange("p (h f) -> p h f", f=FOUT),
                    )
                else:
                    nc.scalar.activation(
                        hp1[:, ic, :, 0:FOUT],
                        ps.rearrange("p (h f) -> p h f", f=FOUT),
                        ACTF.Copy,
                    )

        # ---- software-pipelined emission ----
        emit_pair(0)
        ubs = {0: emit_ub(0)}
        emit_phase_a(0, 4, on_dve=True)
        emit_phase_a(4, NCH)
        emit_hp1q(0)
        for h in range(NH):
            # keep the PE fed and the next head's inputs ready while the
            # vector engine grinds the current head's mask chunks
            if h + 1 < NH:
                if (h + 1) % 2 == 0 and h // 2 + 1 < NPAIR:
                    emit_pair(h // 2 + 1)
                ubs[h + 1] = emit_ub(h + 1)
                if (h + 1) % 2 == 0 and h // 2 + 1 < NPAIR:
                    emit_hp1q(h // 2 + 1)
            emit_head(h, ubs.pop(h))


def build_program(num_devices=8, debug=False):
    nc = bacc.Bacc(
        "TRN2", target_bir_lowering=False, debug=debug, num_devices=num_devices
    )
    hT_d = nc.dram_tensor("hT", [KC, P, N], BF16, kind="ExternalInput").ap()
    w_d = nc.dram_tensor("w_all", [KC, P, NH * FOUT], BF16, kind="ExternalInput").ap()
    aPC_d = nc.dram_tensor("aPC", [NPAIR, P, 4], BF16, kind="ExternalInput").ap()
    aRep_d = nc.dram_tensor("aRep", [P, NPAIR, 2, P], BF16, kind="ExternalInput").ap()
    adjT_d = nc.dram_tensor("adjT", [NCH, P, N], BF16, kind="ExternalInput").ap()
    out_d = nc.dram_tensor("outT", [NH, FOUT + 1, N], F16, kind="ExternalOutput").ap()
    with tile.TileContext(nc) as tc:
        emit(nc, tc, hT_d, w_d, aPC_d, aRep_d, adjT_d, out_d)
    nc.compile()
    return nc


def make_in_maps(h, adj, w, a_src, a_dst):
    """Host-side sharding/layout prep: core c gets batch c."""
    w_all = np.ascontiguousarray(
        w.astype(np.float32).transpose(1, 0, 2).reshape(KC, P, NH * FOUT)
    ).astype(BF16NP)
    # aPC[p]: [128, 4] = (src_A, src_B, dst_A, dst_B) columns for head pair
    # (2p, 2p+1); head A occupies partition rows 0:64, head B rows 64:128.
    aPC = np.zeros((NPAIR, P, 4), dtype=np.float32)
    for p in range(NPAIR):
        aPC[p, 0:FOUT, 0] = a_src[2 * p, :, 0]
        aPC[p, FOUT:P, 1] = a_src[2 * p + 1, :, 0]
        aPC[p, 0:FOUT, 2] = a_dst[2 * p, :, 0]
        aPC[p, FOUT:P, 3] = a_dst[2 * p + 1, :, 0]
    # aRep[f, p, h2, m] = aPC src column, replicated over the 128 moving
    # columns: the stationary that broadcasts src rows to all partitions.
    aRep = np.ascontiguousarray(
        np.broadcast_to(
            aPC[:, :, 0:2].transpose(1, 0, 2)[:, :, :, None], (P, NPAIR, 2, P)
        )
    ).astype(BF16NP)
    aPC = aPC.astype(BF16NP)
    in_maps = []
    for b in range(BS):
        hTb = np.ascontiguousarray(
            h[b].astype(np.float32).T.reshape(KC, P, N)
        ).astype(BF16NP)
        adjTb = np.ascontiguousarray(adj[b].T.reshape(NCH, P, N)).astype(BF16NP)
        in_maps.append(
            {"hT": hTb, "w_all": w_all, "aPC": aPC, "aRep": aRep, "adjT": adjTb}
        )
    return in_maps


def postprocess(raw_outs):
    """raw_outs: list of [NH, FOUT+1, N] per core -> full [BS, NH, N, FOUT]."""
    outT = np.stack([np.asarray(r, dtype=np.float32) for r in raw_outs])
    num = outT[:, :, 0:FOUT, :]
    den = outT[:, :, FOUT : FOUT + 1, :]
    return np.ascontiguousarray((num / den).transpose(0, 1, 3, 2)).astype(np.float32)


_NC_CACHE = {}


def kernel(h, adj, w, a_src, a_dst):
    if "nc" not in _NC_CACHE:
        _NC_CACHE["nc"] = build_program(num_devices=BS)
    nc = _NC_CACHE["nc"]
    in_maps = make_in_maps(h, adj, w, a_src, a_dst)
    res = run_bass_kernel_spmd(nc, in_maps, core_ids=list(range(BS)))
    return postprocess([r["outT"] for r in res.results])
